# revision 9
# baseline (speedup 1.0000x reference)
"""Trainium2 Bass kernel for nn_DeepWarping (8-core data parallel).

Redesign vs the register-offset baseline (34.9us) — three structural moves:

1. Warp via SVD:  T[a] (61 yaw-indexed 36x36 matrices) compresses to
   rank R=12: T[a] ~= sum_h US[a,h] * C_h  (warped rel err ~1e-4; bf16
   inputs dominate at ~2e-3, vs the 2e-2 gate).  On device the warp is
   ONE static bf16 matmul Z2[p,(h,i)] = inp @ C_h plus a tiny DVE
   select with per-row weights uP[p,h] = poly_D(round(yaw*DEG)/26)
   (degree-14 fit of US at the 61 integer yaws).  No PE registers, no
   dynamic slices, no 316KB bank DMA.

2. Correlation via DFT on the PE (bf16):  W[p,k] = sum_i e1[p,i]
   e2[p,(i+k)%36] = (1/36) sum_n [A cos(2pi nk/36) - B sin(...)],
   A = R1R2+I1I2, B = I1R2-R1I2.  The F-transforms and the inverse
   transform are small static matmuls; the prior row expf[k] and the
   normalizer column sz = sum_k wf[k] are folded into the iDFT
   stationary.  Replaces the [112,1296] DVE multiply + strided reduce
   (3.9us) with two [72,112] products (on GpSimd, off the DVE queue).

3. Row-major tail via one PE transpose:  wfT+sz [37,112] transposes to
   [112,37]; then logpost = ln(wf) - ln(sz) is one ACT Ln + one DVE
   tensor_scalar (normalization is algebraically exact); the pop-vector
   readout runs on DVE views; 1/|v| = Exp(-0.5*Ln(n2)).

Act-table trick: Exp and Ln share table set 6 (natural_log_exp_and_
others) but the stock greedy pass picks per-function first-match sets
and pays 3 x 1.28us ACT_TABLE_LOADs.  _Bacc blanks every set that does
not contain both Exp and Ln (index-preserving) so exactly one load is
emitted.

The degree-15 poly coefficient block is broadcast to 112 partitions by
a K=1 outer-product matmul (ones [1,112] stationary) instead of an
80KB replicated DMA.  Input DMAs split across the two HWDGE queues
(sync + scalar).  Fixed NEFF prologue/epilogue (~9us engine init +
semaphore teardown) dominates the remaining runtime.
"""

import numpy as np
import ml_dtypes

import concourse.bacc as bacc
import concourse.masks as masks
import concourse.bass as bass
import concourse.mybir as mybir
import concourse.tile as tile
from concourse.bass_utils import run_bass_kernel_spmd

NB = 36          # angle bins
NA = 61          # yaw bank size
B, S = 128, 7    # full batch / seq
NCORES = 8
BPC = B // NCORES          # batches per core (16)
P = BPC * S                # (b,s) rows per core (112)
OC = 2 * NB + 2            # 74 output cols
DEG = 57.29577951308232    # 180/pi
R = 10                     # SVD rank of the transform bank
D = 15                     # poly degree+1 for the yaw->US fit

# a36 fp32 [36, 224]: X1T | X2T
# b36 bf16 [36, 688]: Cbank [(h,i)] | inpT | dftA | dftB
C_CB, C_INP, C_DFTA, C_DFTB = 0, R * NB, R * NB + P, R * NB + P + 72
WB36 = R * NB + P + 144
# b72 bf16 [72, 74]: idft1 | idft2
# a112 fp32 [112, 73]: yaw112 | pop2i (ident37 is built on device)
C_YAW, C_POP = 0, 1
W112 = 1 + 2 * NB
# cf fp32 [1, D*R]: poly coef (d-major)

_DT = mybir.dt.float32
_BF = mybir.dt.bfloat16


class _Bacc(bacc.Bacc):
    """Bacc that restricts ACT table selection to sets containing both
    Exp and Ln, so the greedy per-activation chooser cannot alternate
    between an exp-only and an ln-only set (3 table loads -> 1)."""

    def insert_act_table_loads(self):
        import bass_rust as _bass_rust
        from concourse.hw_specs import get_activation_tables

        has_activation = any(
            isinstance(i, mybir.InstActivation)
            for b in self.main_func.blocks
            for i in b.instructions
        )
        if not has_activation:
            return
        need = {mybir.ActivationFunctionType.Exp, mybir.ActivationFunctionType.Ln}
        tables = [
            (name, funcs if need <= funcs else set())
            for name, funcs in get_activation_tables(self.m.arch).items()
        ]
        assert any(funcs for _, funcs in tables), "no table with Exp+Ln"
        _bass_rust.insert_act_table_loads(self, tables)


def _fv(base, dims):
    """View of an SBUF/PSUM tile with custom free-dim (step,count) pairs."""
    return bass.AP(
        tensor=base.tensor,
        offset=base.offset,
        ap=[list(base.ap[0])] + [list(d) for d in dims],
    )


def _emit(nc):
    dt = _DT
    bf = _BF
    d_a36 = nc.dram_tensor("a36", [NB, 2 * P], bf, kind="ExternalInput")
    d_b36 = nc.dram_tensor("b36", [NB, WB36], bf, kind="ExternalInput")
    d_b72 = nc.dram_tensor("b72", [72, 74], bf, kind="ExternalInput")
    d_a112 = nc.dram_tensor("a112", [P, W112], dt, kind="ExternalInput")
    d_cf = nc.dram_tensor("cf", [1, D * R], dt, kind="ExternalInput")
    d_out = nc.dram_tensor("out", [P, OC], dt, kind="ExternalOutput")

    alu = mybir.AluOpType
    act = mybir.ActivationFunctionType
    X = mybir.AxisListType.X

    with tile.TileContext(nc) as tc:
        with (
            tc.tile_pool(name="sb", bufs=1) as sb,
            tc.tile_pool(name="ps", bufs=1, space="PSUM") as ps,
        ):
            a36 = sb.tile([NB, 2 * P], bf, tag="a36")
            b36 = sb.tile([NB, WB36], bf, tag="b36")
            b72 = sb.tile([72, 74], bf, tag="b72")
            a112 = sb.tile([P, W112], dt, tag="a112")
            cf = sb.tile([1, D * R], dt, tag="cf")
            ones1 = sb.tile([1, P], dt, tag="ones1")
            t12e = sb.tile([NB, 2 * P], bf, tag="t12e")
            fsb = sb.tile([72, 336], bf, tag="fsb")
            dyw = sb.tile([P, 1], dt, tag="dyw")
            dyi = sb.tile([P, 1], mybir.dt.int32, tag="dyi")
            dyf = sb.tile([P, 1], dt, tag="dyf")
            powr = sb.tile([P, D], dt, tag="powr")
            q3 = sb.tile([P, D * R], dt, tag="q3")
            uP = sb.tile([P, R], dt, tag="uP")
            selq = sb.tile([P, R * NB], dt, tag="selq")
            p1 = sb.tile([72, P], bf, tag="p1")
            p2 = sb.tile([72, P], bf, tag="p2")
            asmT = sb.tile([37, P], dt, tag="asmT")
            lnrow = sb.tile([P, 37], dt, tag="lnrow")
            prdv = sb.tile([P, 2 * NB], dt, tag="prdv")
            vecu = sb.tile([P, 2], dt, tag="vecu")
            sqx = sb.tile([P, 1], dt, tag="sqx")
            n2 = sb.tile([P, 1], dt, tag="n2")
            rn = sb.tile([P, 1], dt, tag="rn")
            lnn = sb.tile([P, 1], dt, tag="lnn")
            vclip = sb.tile([P, 2], dt, tag="vclip")
            eye37 = sb.tile([37, 37], dt, tag="eye37")
            outb = sb.tile([P, OC], dt, tag="outb")
            z2 = ps.tile([P, R * NB], dt, tag="z2")
            cfps = ps.tile([P, D * R], dt, tag="cfps")
            fps = ps.tile([72, 336], dt, tag="fps")
            wfT = ps.tile([37, P], dt, tag="wfT")
            rowP = ps.tile([P, 37], dt, tag="rowP")

            # ---- loads: split across the two HWDGE queues ----
            nc.sync.dma_start(a36[:], d_a36[:])
            nc.sync.dma_start(b36[:], d_b36[:])
            nc.sync.dma_start(b72[:], d_b72[:])
            nc.scalar.dma_start(a112[:], d_a112[:])
            nc.scalar.dma_start(cf[:], d_cf[:])
            nc.gpsimd.memset(ones1[:], 1.0)
            masks.make_identity(nc, eye37[:])
            nc.gpsimd.memset(powr[:, 0:1], 1.0)

            # ---- broadcast poly coef row to 112 partitions (K=1 outer) ----
            nc.tensor.matmul(cfps[:], ones1[:], cf[:], start=True, stop=True)

            # ---- warp: Z2[p,(h,i)] = inp @ C_h (bf16 PE) ----
            nc.tensor.matmul(z2[:], b36[:, C_INP:C_INP + P],
                             b36[:, C_CB:C_CB + R * NB], start=True, stop=True)

            # ---- exp(ll1T | ll2T) on ACT, then DFT matmuls on PE ----
            nc.scalar.activation(t12e[:], a36[:], act.Exp)
            nc.tensor.matmul(fps[:, 0:224], b36[:, C_DFTA:C_DFTA + 72],
                             t12e[:], start=True, stop=True)
            nc.tensor.matmul(fps[:, 224:336], b36[:, C_DFTB:C_DFTB + 72],
                             t12e[:, P:2 * P], start=True, stop=True)

            # ---- yaw -> poly powers -> uP = powr @ coef (DVE) ----
            # t = (round(yaw*DEG + 30) - 30)/26; shift keeps the cast
            # argument positive (f32->i32 copy rounds-to-nearest on HW).
            nc.vector.tensor_scalar(dyw[:], a112[:, C_YAW:C_YAW + 1],
                                    DEG, 30.0, alu.mult, alu.add)
            nc.vector.tensor_copy(dyi[:], dyw[:])
            nc.vector.tensor_copy(dyf[:], dyi[:])
            nc.vector.tensor_scalar(powr[:, 1:2], dyf[:], 1.0 / 26.0,
                                    -30.0 / 26.0, alu.mult, alu.add)
            # powr[:,d] = t^d via one geometric prefix scan (state *= t)
            nc.vector.tensor_tensor_scan(powr[:, 2:D],
                                         _fv(powr[:, 1:2], [[0, D - 2]]),
                                         _fv(powr[:, 1:2], [[0, D - 2]]),
                                         powr[:, 1:2], alu.mult, alu.bypass)
            # ---- correlation: P1/P2 products, iDFT accumulate ----
            # (high priority: this chain gates the whole logpost/vec tail,
            # while the warp select above only gates the early store)
            with tc.high_priority():
                nc.scalar.copy(fsb[:], fps[:])
                nc.vector.tensor_mul(p1[:], fsb[:, 0:P], fsb[:, P:2 * P])
                nc.vector.tensor_mul(p2[:], fsb[:, 0:P],
                                     fsb[:, 2 * P:3 * P])
                nc.tensor.matmul(wfT[:], b72[:, 0:37], p1[:],
                                 start=True, stop=False)
                nc.tensor.matmul(wfT[:], b72[:, 37:74], p2[:],
                                 start=False, stop=True)

            # q3[p,(d,h)] = powr[p,d] * coef[d,h]; reduce over d -> uP
            nc.vector.tensor_mul(q3[:].rearrange("p (d h) -> p d h", d=D),
                                 _fv(powr[:], [[1, D], [0, R]]),
                                 _fv(cfps[:], [[R, D], [1, R]]))
            nc.vector.reduce_sum(uP[:], _fv(q3[:], [[1, R], [R, D]]), axis=X)

            # ---- warp select: warped = sum_h uP[p,h] * Z2[p,(h,i)] ----
            nc.vector.tensor_mul(selq[:].rearrange("p (h i) -> p h i", h=R),
                                 _fv(z2[:], [[NB, R], [1, NB]]),
                                 _fv(uP[:], [[1, R], [0, NB]]))
            nc.vector.reduce_sum(outb[:, 0:NB],
                                 _fv(selq[:], [[1, NB], [NB, R]]), axis=X)

            # ---- transpose wf+sz to row layout ----
            nc.scalar.copy(asmT[:], wfT[:])
            nc.tensor.transpose(rowP[:], asmT[:], eye37[:])

            # ---- logpost = ln(wf) - ln(sz) ----
            nc.scalar.activation(lnrow[:], rowP[:], act.Ln)
            nc.vector.tensor_scalar(outb[:, NB + 2:OC], lnrow[:, 0:NB],
                                    lnrow[:, 36:37], None, alu.subtract)

            # ---- population vector readout + normalize + clip ----
            nc.vector.tensor_mul(prdv[:].rearrange("p (k c) -> p k c", k=NB),
                                 _fv(rowP[:, 0:NB], [[1, NB], [0, 2]]),
                                 _fv(a112[:, C_POP:C_POP + 2 * NB],
                                     [[2, NB], [1, 2]]))
            nc.vector.reduce_sum(vecu[:], _fv(prdv[:], [[1, 2], [2, NB]]),
                                 axis=X)
            # vec_x += 1e-8 * sz (the reference adds eps before normalizing)
            nc.vector.scalar_tensor_tensor(vecu[:, 0:1], rowP[:, 36:37],
                                           1e-8, vecu[:, 0:1],
                                           alu.mult, alu.add)
            nc.vector.tensor_mul(sqx[:], vecu[:, 0:1], vecu[:, 0:1])
            nc.vector.scalar_tensor_tensor(n2[:], vecu[:, 1:2],
                                           vecu[:, 1:2], sqx[:],
                                           alu.mult, alu.add)
            nc.scalar.activation(lnn[:], n2[:], act.Ln)
            nc.scalar.activation(rn[:], lnn[:], act.Exp, scale=-0.5)
            nc.vector.tensor_scalar(vclip[:], vecu[:], rn[:, :1], 1.0,
                                    alu.mult, alu.min)
            nc.vector.tensor_scalar(outb[:, NB:NB + 2], vclip[:], -1.0,
                                    None, alu.max)
            nc.sync.dma_start(d_out[:, NB:OC], outb[:, NB:OC])

            # ---- stores: warped early, vec+logpost at the end ----
            nc.sync.dma_start(d_out[:, 0:NB], outb[:, 0:NB])

    return nc


_NC_CACHE = {}


def _get_nc():
    nc = _NC_CACHE.get(0)
    if nc is None:
        nc = _emit(_Bacc(None, target_bir_lowering=False))
        nc.compile()
        _NC_CACHE[0] = nc
    return nc


_CONST_CACHE = {}


def _consts(transform_matrices, logprior_rotate_matrix, population_vector):
    """Host-side stationaries derived from the (constant) model tensors."""
    key = (transform_matrices.tobytes()[:256], logprior_rotate_matrix.tobytes()[:64])
    hit = _CONST_CACHE.get(key)
    if hit is not None:
        return hit
    f32 = np.float32
    bf16 = ml_dtypes.bfloat16
    T = np.asarray(transform_matrices, np.float64)
    M = np.asarray(logprior_rotate_matrix, np.float64)
    pop = np.asarray(population_vector, f32)

    U, Sv, Vt = np.linalg.svd(T.reshape(NA, -1), full_matrices=False)
    US = U[:, :R] * Sv[:R]
    angles = np.arange(-30, 31)
    V = np.vander(angles / 26.0, D, increasing=True)
    coef, *_ = np.linalg.lstsq(V, US, rcond=None)          # [D, R]
    Cb = Vt[:R].reshape(R, NB, NB)
    cbank = np.transpose(Cb, (2, 0, 1)).reshape(NB, R * NB)  # [j,(h,i)]

    n = np.arange(NB)
    ang = 2 * np.pi * np.outer(n, n) / NB
    Cm, Sm = np.cos(ang), np.sin(ang)
    dftA = np.concatenate([Cm, Sm], 1)                      # [i, 72]
    dftB = np.concatenate([Sm, Cm], 1)
    expf = np.exp(M[0, :])
    KK = Cm * expf[None, :] / NB                            # cos(2pi nk/36)*expf/36
    SS = Sm * expf[None, :] / NB
    id1 = np.concatenate([KK, KK], 0)                       # [72, 36]
    id2 = np.concatenate([SS, -SS], 0)
    id1 = np.concatenate([id1, id1.sum(1, keepdims=True)], 1)
    id2 = np.concatenate([id2, id2.sum(1, keepdims=True)], 1)
    b72 = np.concatenate([id1, id2], 1).astype(bf16)        # [72, 74]

    cbank_dft = np.concatenate(
        [cbank, dftA, dftB], axis=1).astype(bf16)           # [36, 432+144]
    a112c = np.tile(pop.T.reshape(-1), (P, 1)).astype(f32)  # pop2i
    out = (cbank_dft, b72, a112c, coef.reshape(1, -1).astype(f32))
    _CONST_CACHE[key] = out
    return out


def _in_maps(loglikelihood1, loglikelihood2, inp, yaw,
             transform_matrices, logprior_rotate_matrix, template_log,
             population_vector):
    f32 = np.float32
    bf16 = ml_dtypes.bfloat16
    cbank_dft, b72, a112c, cf = _consts(
        np.asarray(transform_matrices, f32),
        np.asarray(logprior_rotate_matrix, f32),
        np.asarray(population_vector, f32))
    ll1 = np.ascontiguousarray(loglikelihood1, f32)
    ll2 = np.ascontiguousarray(loglikelihood2, f32)
    inp = np.ascontiguousarray(inp, f32)
    yaw = np.ascontiguousarray(yaw, f32)

    maps = []
    for c in range(NCORES):
        bs = slice(BPC * c, BPC * (c + 1))
        x1t = ll1[bs].reshape(P, NB).T                      # [36, 112]
        x2t = ll2[bs].reshape(P, NB).T
        a36 = np.concatenate([x1t, x2t], axis=1).astype(bf16)
        b36 = np.concatenate(
            [cbank_dft[:, :R * NB],
             inp[bs].reshape(P, NB).T.astype(bf16),
             cbank_dft[:, R * NB:]], axis=1)
        a112 = np.concatenate(
            [np.repeat(yaw[bs], S).reshape(P, 1), a112c], axis=1)
        maps.append({
            "a36": np.ascontiguousarray(a36),
            "b36": np.ascontiguousarray(b36),
            "b72": b72,
            "a112": np.ascontiguousarray(a112),
            "cf": cf,
        })
    return maps


def run(trace=False, **inputs):
    """Run on 8 NeuronCores; returns (full_output, exec_time_ns_or_None)."""
    nc = _get_nc()
    maps = _in_maps(**inputs)
    res = run_bass_kernel_spmd(nc, maps, list(range(NCORES)), trace=trace)
    parts = [res.results[c]["out"].reshape(BPC, S, OC) for c in range(NCORES)]
    out = np.concatenate(parts, axis=0).astype(np.float32)
    return out, res.exec_time_ns


def kernel(**inputs):
    return run(trace=False, **inputs)[0]
